# revision 3
# baseline (speedup 1.0000x reference)
"""Trainium2 Bass kernel for nn_CrossCompressUnit.

Math (per batch row b):
    v_out[b] = v[b] * (e[b]@w_vv) + e[b] * (v[b]@w_ev) + bias_v
    e_out[b] = v[b] * (e[b]@w_ve) + e[b] * (v[b]@w_ee) + bias_e

Strategy:
  - Pure data parallel over 8 cores on the batch dim (B=131072 -> 16384/core).
  - Host passes inputs TRANSPOSED ([DIM, B_core]) so the contraction dim
    (DIM) sits on SBUF partitions: the dot products run on the TensorEngine
    with the tiny packed weight matrix [128, 2] as the stationary operand.
  - TensorEngine also transposes the data tiles back to batch-major
    ([128 batch, 256 dim]) into PSUM; ScalarE/VectorE consume PSUM directly:
        t1    = ACT (psum_v * s_vv)           (per-partition scale)
        v_out = DVE scalar_tensor_tensor((psum_e * s_ev) + t1)
    and symmetrically for e_out. Outputs are written batch-major, so no
    host-side un-transpose is needed.
  - Explicit raw-Bass semaphore pipeline (the Tile scheduler emits
    multi-wait sync_infos this walrus build rejects). DMA completions from
    different HW queues are unordered, so every DMA stream has its own
    semaphore and each transfer carries a descriptor-level wait on its
    predecessor (completion chaining) to make cumulative thresholds sound.
"""

import numpy as np

import concourse.bass as bass
import concourse.mybir as mybir
from concourse.bass_utils import run_bass_kernel_spmd

F32 = mybir.dt.float32
DIM = 256
P = 128
N_CORES = 8
F = 512           # batch columns per block
SUB = F // P      # 128-row sub-blocks per block

LAST_RESULT = None  # BassKernelResults of the most recent kernel() call (for test harness)


def build_nc(B_core, rep=1, with_bias=False):
    NB = B_core // F
    G = NB * rep
    nc = bass.Bass()

    vT = nc.dram_tensor("vT", [DIM, B_core], F32, kind="ExternalInput")
    eT = nc.dram_tensor("eT", [DIM, B_core], F32, kind="ExternalInput")
    wp = nc.dram_tensor("wpack", [DIM, 4], F32, kind="ExternalInput")
    idn = nc.dram_tensor("ident", [P, P], F32, kind="ExternalInput")
    if with_bias:
        bvr = nc.dram_tensor("brep_v", [P, DIM], F32, kind="ExternalInput")
        ber = nc.dram_tensor("brep_e", [P, DIM], F32, kind="ExternalInput")
    vo = nc.dram_tensor("v_out", [B_core, DIM], F32, kind="ExternalOutput")
    eo = nc.dram_tensor("e_out", [B_core, DIM], F32, kind="ExternalOutput")

    n_setup = 4 if with_bias else 2
    mult = mybir.AluOpType.mult
    add = mybir.AluOpType.add
    Copy = mybir.ActivationFunctionType.Copy

    from contextlib import ExitStack
    with ExitStack() as ctx:
        ent = ctx.enter_context
        # SBUF
        in_t = ent(nc.sbuf_tensor("in_t", [P, 2, 4, F], F32))      # [e0,e1,v0,v1]
        w_sb = ent(nc.sbuf_tensor("w_sb", [P, 2, 4], F32))
        id_sb = ent(nc.sbuf_tensor("id_sb", [P, P], F32))
        dots_e_sb = ent(nc.sbuf_tensor("dots_e_sb", [2, 2, F], F32))
        dots_v_sb = ent(nc.sbuf_tensor("dots_v_sb", [2, 2, F], F32))
        dT_sb = ent(nc.sbuf_tensor("dT_sb", [P, 2, 4 * SUB], F32))
        t1_sb = ent(nc.sbuf_tensor("t1_sb", [P, 3, DIM], F32))
        t3_sb = ent(nc.sbuf_tensor("t3_sb", [P, 3, DIM], F32))
        vout_sb = ent(nc.sbuf_tensor("vout_sb", [P, 2, SUB, DIM], F32))
        eout_sb = ent(nc.sbuf_tensor("eout_sb", [P, 2, SUB, DIM], F32))
        if with_bias:
            bv_sb = ent(nc.sbuf_tensor("bv_sb", [P, DIM], F32))
            be_sb = ent(nc.sbuf_tensor("be_sb", [P, DIM], F32))
        # PSUM (each tensor gets whole banks; 1+1+1+1+3 = 7 of 8)
        p_de = ent(nc.psum_tensor("p_de", [2, F], F32))
        p_dv = ent(nc.psum_tensor("p_dv", [2, F], F32))
        p_dT0 = ent(nc.psum_tensor("p_dT0", [P, 4 * SUB], F32))
        p_dT1 = ent(nc.psum_tensor("p_dT1", [P, 4 * SUB], F32))
        p_data = ent(nc.psum_tensor("p_data", [P, 3, 2 * DIM], F32))
        p_dT = [p_dT0, p_dT1]
        # semaphores
        setup_sem = ent(nc.semaphore("setup_sem"))
        ld = [ent(nc.semaphore(f"ld{t}")) for t in range(4)]   # e0,e1,v0,v1 chains
        st_v = ent(nc.semaphore("st_v"))
        st_e = ent(nc.semaphore("st_e"))
        dots_pe = ent(nc.semaphore("dots_pe"))
        dots_cp = ent(nc.semaphore("dots_cp"))
        dT_pe = ent(nc.semaphore("dT_pe"))
        dT_cp = ent(nc.semaphore("dT_cp"))
        data_pe = ent(nc.semaphore("data_pe"))
        mul_sem = ent(nc.semaphore("mul_sem"))
        stt_v = ent(nc.semaphore("stt_v"))
        t13_sem = ent(nc.semaphore("t13_sem"))
        stt_e = ent(nc.semaphore("stt_e"))
        block = ent(nc.Block())

        def w(val):
            """skip trivially-satisfied waits"""
            return val > 0

        @block.sync
        def _(sp):
            # one-time setup loads (threshold == total, so no ordering race)
            sp.dma_start(out=w_sb[:, :, :], in_=wp.rearrange("(c p) j -> p c j", p=P)).then_inc(setup_sem, 16)
            sp.dma_start(out=id_sb[:, :], in_=idn[:, :]).then_inc(setup_sem, 16)
            if with_bias:
                sp.dma_start(out=bv_sb[:, :], in_=bvr[:, :]).then_inc(setup_sem, 16)
                sp.dma_start(out=be_sb[:, :], in_=ber[:, :]).then_inc(setup_sem, 16)

            def issue_stores(gs):
                ks = gs % NB
                b2 = gs % 2
                dview_v = vo[ks * F:(ks + 1) * F, :].rearrange("(i p) d -> p i d", p=P)
                dview_e = eo[ks * F:(ks + 1) * F, :].rearrange("(i p) d -> p i d", p=P)
                sp.wait_ge(stt_v, SUB * (gs + 1))
                (sp.dma_start(out=dview_v, in_=vout_sb[:, b2, :, :])
                   ._wait_ge(st_v, 16 * gs).then_inc(st_v, 16))
                sp.wait_ge(stt_e, SUB * (gs + 1))
                (sp.dma_start(out=dview_e, in_=eout_sb[:, b2, :, :])
                   ._wait_ge(st_e, 16 * gs).then_inc(st_e, 16))

            for g in range(G):
                k = g % NB
                k2 = g % 2
                c0 = k * F
                # WAR: in-tile buffer reused from block g-2; PE finished it
                # when its last data-transpose bumped data_pe to 4*(g-1).
                if w(4 * (g - 1)):
                    sp.wait_ge(data_pe, 4 * (g - 1))
                srcs = [eT[0:P, c0:c0 + F], eT[P:DIM, c0:c0 + F],
                        vT[0:P, c0:c0 + F], vT[P:DIM, c0:c0 + F]]
                for t in range(4):
                    ins = sp.dma_start(out=in_t[:, k2, t, :], in_=srcs[t])
                    if w(16 * g):
                        ins._wait_ge(ld[t], 16 * g)    # chain: completions in order
                    ins.then_inc(ld[t], 16)
                if g >= 1:
                    issue_stores(g - 1)
            issue_stores(G - 1)

        @block.tensor
        def _(pe):
            pe.wait_ge(setup_sem, 16 * n_setup)
            for g in range(G):
                k2 = g % 2
                # dots: psum_de = [w_vv w_ve]^T @ eT ; psum_dv = [w_ev w_ee]^T @ vT
                if w(g):
                    pe.wait_ge(dots_cp, g)      # WAR on dots banks
                pe.wait_ge(ld[0], 16 * (g + 1))
                pe.wait_ge(ld[1], 16 * (g + 1))
                nc.tensor.matmul(p_de[:, :], w_sb[:, 0, 0:2], in_t[:, k2, 0, :], start=True, stop=False)
                nc.tensor.matmul(p_de[:, :], w_sb[:, 1, 0:2], in_t[:, k2, 1, :], start=False, stop=True)
                pe.wait_ge(ld[2], 16 * (g + 1))
                pe.wait_ge(ld[3], 16 * (g + 1))
                nc.tensor.matmul(p_dv[:, :], w_sb[:, 0, 2:4], in_t[:, k2, 2, :], start=True, stop=False)
                nc.tensor.matmul(p_dv[:, :], w_sb[:, 1, 2:4], in_t[:, k2, 3, :], start=False, stop=True).then_inc(dots_pe, 1)
                # transpose dots [4, F] -> SUB x [128, 4]
                pe.wait_ge(dots_cp, g + 1)
                if w(g - 1):
                    pe.wait_ge(dT_cp, g - 1)    # WAR on dT psum buffer
                for i in range(SUB):
                    nc.tensor.transpose(
                        p_dT[k2][:, 4 * i:4 * i + 2],
                        dots_e_sb[0:2, k2, P * i:P * (i + 1)],
                        id_sb[0:2, 0:2],
                    )
                    ins = nc.tensor.transpose(
                        p_dT[k2][:, 4 * i + 2:4 * i + 4],
                        dots_v_sb[0:2, k2, P * i:P * (i + 1)],
                        id_sb[0:2, 0:2],
                    )
                ins.then_inc(dT_pe, 1)
                # transpose data tiles into batch-major PSUM
                for i in range(SUB):
                    n = SUB * g + i
                    b = n % 3
                    if n >= 3:
                        m = n - 3
                        if not with_bias:
                            pe.wait_ge(mul_sem, 2 * m + 2)   # WAR on p_data bank b
                        pe.wait_ge(stt_v, m + 1)
                        pe.wait_ge(stt_e, m + 1)
                    nc.tensor.transpose(p_data[:, b, 0:P], in_t[:, k2, 2, P * i:P * (i + 1)], id_sb[:, :])
                    nc.tensor.transpose(p_data[:, b, P:DIM], in_t[:, k2, 3, P * i:P * (i + 1)], id_sb[:, :])
                    nc.tensor.transpose(p_data[:, b, DIM:DIM + P], in_t[:, k2, 0, P * i:P * (i + 1)], id_sb[:, :])
                    nc.tensor.transpose(p_data[:, b, DIM + P:2 * DIM], in_t[:, k2, 1, P * i:P * (i + 1)], id_sb[:, :]).then_inc(data_pe, 1)

        @block.scalar
        def _(act):
            for g in range(G):
                k2 = g % 2
                act.wait_ge(dots_pe, g + 1)
                if w(g - 1):
                    act.wait_ge(dT_pe, g - 1)   # WAR on dots_sb buffer
                nc.scalar.copy(dots_e_sb[0:2, k2, :], p_de[:, :])
                nc.scalar.copy(dots_v_sb[0:2, k2, :], p_dv[:, :]).then_inc(dots_cp, 1)
                if not with_bias:
                    act.wait_ge(dT_cp, g + 1)   # scales ready in SBUF
                    for i in range(SUB):
                        n = SUB * g + i
                        s = n % 3
                        if n >= 3:
                            act.wait_ge(stt_v, n - 2)   # WAR t1 slot (DVE read done)
                            act.wait_ge(stt_e, n - 2)   # WAR t3 slot
                        act.wait_ge(data_pe, n + 1)
                        nc.scalar.activation(t1_sb[:, s, :], p_data[:, s, 0:DIM], Copy,
                                             scale=dT_sb[:, k2, 4 * i + 0:4 * i + 1]).then_inc(mul_sem, 1)
                        nc.scalar.activation(t3_sb[:, s, :], p_data[:, s, DIM:2 * DIM], Copy,
                                             scale=dT_sb[:, k2, 4 * i + 3:4 * i + 4]).then_inc(mul_sem, 1)

        @block.vector
        def _(dve):
            if with_bias:
                dve.wait_ge(setup_sem, 16 * n_setup)
            for g in range(G):
                k2 = g % 2
                dve.wait_ge(dT_pe, g + 1)
                if not with_bias and w(8 * (g - 1)):
                    dve.wait_ge(mul_sem, 8 * (g - 1))  # WAR dT_sb buffer (ACT readers)
                nc.vector.tensor_copy(out=dT_sb[:, k2, :], in_=p_dT[k2][:, :]).then_inc(dT_cp, 1)
                if with_bias:
                    # same-engine RAW: STTs below read dT_sb written just above
                    dve.wait_ge(dT_cp, g + 1)
                # WAR: vout/eout block buffer stored out two blocks ago
                if w(16 * (g - 1)):
                    dve.wait_ge(st_v, 16 * (g - 1))
                    dve.wait_ge(st_e, 16 * (g - 1))
                for i in range(SUB):
                    n = SUB * g + i
                    s = n % 3
                    dve.wait_ge(data_pe, n + 1)
                    if with_bias:
                        # t1 = v*s_vv + bias_v ; v_out = e*s_ev + t1
                        nc.vector.scalar_tensor_tensor(
                            t1_sb[:, s, :], p_data[:, s, 0:DIM],
                            dT_sb[:, k2, 4 * i + 0:4 * i + 1], bv_sb[:, :], mult, add).then_inc(t13_sem, 1)
                        nc.vector.scalar_tensor_tensor(
                            t3_sb[:, s, :], p_data[:, s, DIM:2 * DIM],
                            dT_sb[:, k2, 4 * i + 3:4 * i + 4], be_sb[:, :], mult, add).then_inc(t13_sem, 1)
                        dve.wait_ge(t13_sem, 2 * n + 1)
                        nc.vector.scalar_tensor_tensor(
                            vout_sb[:, k2, i, :], p_data[:, s, DIM:2 * DIM],
                            dT_sb[:, k2, 4 * i + 2:4 * i + 3], t1_sb[:, s, :], mult, add).then_inc(stt_v, 1)
                        dve.wait_ge(t13_sem, 2 * n + 2)
                        nc.vector.scalar_tensor_tensor(
                            eout_sb[:, k2, i, :], p_data[:, s, 0:DIM],
                            dT_sb[:, k2, 4 * i + 1:4 * i + 2], t3_sb[:, s, :], mult, add).then_inc(stt_e, 1)
                    else:
                        dve.wait_ge(mul_sem, 2 * n + 2)
                        nc.vector.scalar_tensor_tensor(
                            vout_sb[:, k2, i, :], p_data[:, s, DIM:2 * DIM],
                            dT_sb[:, k2, 4 * i + 2:4 * i + 3], t1_sb[:, s, :], mult, add).then_inc(stt_v, 1)
                        dve.wait_ge(mul_sem, 2 * n + 2)
                        nc.vector.scalar_tensor_tensor(
                            eout_sb[:, k2, i, :], p_data[:, s, 0:DIM],
                            dT_sb[:, k2, 4 * i + 1:4 * i + 2], t3_sb[:, s, :], mult, add).then_inc(stt_e, 1)

    return nc


def _prep_core_inputs(v, e, wpack, ident, b_v, b_e, with_bias, c, B_core):
    lo = c * B_core
    m = {
        "vT": np.ascontiguousarray(v[lo:lo + B_core].T),
        "eT": np.ascontiguousarray(e[lo:lo + B_core].T),
        "wpack": wpack,
        "ident": ident,
    }
    if with_bias:
        m["brep_v"] = np.ascontiguousarray(np.broadcast_to(b_v, (P, DIM)))
        m["brep_e"] = np.ascontiguousarray(np.broadcast_to(b_e, (P, DIM)))
    return m


def kernel(v, e, w_vv, w_ev, w_ve, w_ee, bias_v, bias_e):
    v = np.asarray(v, dtype=np.float32)
    e = np.asarray(e, dtype=np.float32)
    w_vv = np.asarray(w_vv, dtype=np.float32)
    w_ev = np.asarray(w_ev, dtype=np.float32)
    w_ve = np.asarray(w_ve, dtype=np.float32)
    w_ee = np.asarray(w_ee, dtype=np.float32)
    bias_v = np.asarray(bias_v, dtype=np.float32)
    bias_e = np.asarray(bias_e, dtype=np.float32)

    B = v.shape[0]
    B_core = B // N_CORES
    with_bias = bool(np.any(bias_v) or np.any(bias_e))

    wpack = np.ascontiguousarray(np.stack([w_vv, w_ve, w_ev, w_ee], axis=1))
    ident = np.eye(P, dtype=np.float32)

    nc = build_nc(B_core, rep=1, with_bias=with_bias)
    in_maps = [
        _prep_core_inputs(v, e, wpack, ident, bias_v, bias_e, with_bias, c, B_core)
        for c in range(N_CORES)
    ]
    res = run_bass_kernel_spmd(nc, in_maps, core_ids=list(range(N_CORES)), trace=False)
    global LAST_RESULT
    LAST_RESULT = res
    v_out = np.concatenate([res.results[c]["v_out"] for c in range(N_CORES)], axis=0)
    e_out = np.concatenate([res.results[c]["e_out"] for c in range(N_CORES)], axis=0)
    return (v_out, e_out)

